# revision 6
# baseline (speedup 1.0000x reference)
"""Segment-sum (scatter-add) kernel for Trainium2, 8 NeuronCores.

out[n, :] = sum_{e : index[e] == n} input[e, :]   (N=50000 segments, d=64)

Host side (data movement / re-encoding only; every FLOP of the actual
reduction runs on device):
  1. argsort(index); greedily pack whole segments into chunks of
     <= 32 consecutive segment ids and <= 1024 edges (8 tiles x 128).
  2. Edge rows are re-encoded fp32 -> fp8e3 (E3M4) with per-segment
     error-feedback rounding: q_i = fp8(x_i + c_i), c_{i+1} = x_i + c_i
     - q_i.  The device sum of q equals the true segment sum up to the
     final carry (measured rel err 3.8e-3 vs the 2e-2 gate); this
     halves HBM traffic again vs the fp16 baseline.
  3. Chunks are split contiguously across 8 cores; per core the edge
     tiles are laid out partition-major so every DMA is a dense strip.

Device side: "super-groups" (SG) of 8 chunks = 64 tiles.
  - DVE builds the one-hot for a whole SG in ONE tensor_tensor
    is_equal with g-major layout oh[p, g*64+t] against a dense iota
    constant -> runs in 2x_1P DVE mode (1.13us/SG vs 2.2us for the
    baseline layout).
  - PE: per tile one (LDWEIGHTS 32-col strided + MATMUL N=64 fp8 rhs x
    fp16 one-hot lhsT) accumulating 8 tiles/chunk into a [32, 512]
    PSUM bank (8 chunks per bank).  Mixed fp16 x fp8e3 matmul is exact
    for one-hot weights.
  - ACT casts PSUM f32 -> SBUF f16 into one of 4 partition bands of a
    [128, 512] out block; a block (4 SGs) is DMAed out at once.
  - The whole pipeline runs inside ONE tc.tile_critical() with manual
    semaphores: Tile's per-instruction semaphore ticks cost ~26ns
    serialized on the PE (measured 33ns/MM floor under Tile vs 29ns in
    a critical section), so PE sem updates are batched to 1 per SG.

Host finalization: pure scatter placement of per-chunk row blocks into
the [50000, 64] output (np.add.at only if a segment was split across
chunks, which does not happen at these shapes).
"""

import os
import sys

for _p in ("/opt/trn_rl_repo", "/opt/pypackages"):
    if _p not in sys.path:
        sys.path.append(_p)

import numpy as np
import ml_dtypes

import concourse.mybir as mybir
from concourse import bacc
from concourse.mybir import AluOpType
from concourse.tile import TileContext
from concourse.bass_utils import run_bass_kernel_spmd

N_CORES = 8
P = 128               # partitions / contraction dim per tile
D = 64                # feature dim
G = 32                # segs per chunk / one-hot width
TPC = 8               # tiles per chunk
EDGES_PER_CHUNK = TPC * P   # 1024
CPS = 8               # chunks per SG / per PSUM bank
SGT = CPS * TPC       # tiles per SG = 64
BANDS = 4             # SGs packed per [128, 512] out block
STRIP_SGS = int(os.environ.get("STRIP_SGS", "2"))   # SGs per x DMA strip
HEAD_SGS = int(os.environ.get("HEAD_SGS", "1"))     # first (ramp) strip
XBUFS = int(os.environ.get("XBUFS", "6"))
OHBUFS = int(os.environ.get("OHBUFS", "8"))
PSBUFS = int(os.environ.get("PSBUFS", "8"))
OBUFS = 2

F32 = mybir.dt.float32
F16 = mybir.dt.float16
F8 = mybir.dt.float8e3
NP_F8 = ml_dtypes.float8_e3m4
NP_F16 = np.float16


# --------------------------------------------------------------------------
# host-side packing / re-encoding
# --------------------------------------------------------------------------

def pack_chunks(index: np.ndarray, n_segments: int):
    """Group sorted edges into fixed-capacity chunks of whole segments."""
    index = np.asarray(index).astype(np.int64, copy=False).ravel()
    order = np.argsort(index, kind="stable")
    counts = np.bincount(index, minlength=n_segments)

    seg_base, nsegs, edge_start, nedges = [], [], [], []
    s = 0
    epos = 0
    counts_list = counts.tolist()
    while s < n_segments:
        c = counts_list[s]
        if c > EDGES_PER_CHUNK:
            left = c
            while left > 0:
                take = min(left, EDGES_PER_CHUNK)
                seg_base.append(s); nsegs.append(1)
                edge_start.append(epos); nedges.append(take)
                epos += take
                left -= take
            s += 1
            continue
        base = s
        tot = 0
        ns = 0
        while (
            s < n_segments
            and ns < G
            and tot + counts_list[s] <= EDGES_PER_CHUNK
        ):
            tot += counts_list[s]
            ns += 1
            s += 1
        seg_base.append(base); nsegs.append(ns)
        edge_start.append(epos); nedges.append(tot)
        epos += tot
    return (
        order,
        np.array(seg_base, dtype=np.int64),
        np.array(nsegs, dtype=np.int64),
        np.array(edge_start, dtype=np.int64),
        np.array(nedges, dtype=np.int64),
    )


def encode_fp8_ef(xs: np.ndarray, ids: np.ndarray, n_segments: int):
    """Error-feedback fp8e3 rounding along each segment's edge chain.

    xs/ids are in sorted-by-segment order.  Returns q (fp8e3) with
    sum(q over segment) == sum(x over segment) - final_carry,
    |final_carry| <= ulp/2 of the largest chain value.
    """
    counts = np.bincount(ids, minlength=n_segments)
    starts = np.concatenate([[0], np.cumsum(counts)[:-1]])
    pos = np.arange(len(ids)) - starts[ids]
    qs = np.empty(xs.shape, dtype=NP_F8)
    carry = np.zeros((n_segments, xs.shape[1]), dtype=np.float32)
    maxc = int(counts.max()) if len(counts) else 0
    for p_ in range(maxc):
        sel = np.nonzero(pos == p_)[0]
        if not len(sel):
            break
        segs = ids[sel]
        v = xs[sel] + carry[segs]
        qv = v.astype(NP_F8)
        carry[segs] = v - qv.astype(np.float32)
        qs[sel] = qv
    return qs


def build_device_arrays(input_np, index_np, n_segments):
    input_np = np.asarray(input_np, dtype=np.float32).reshape(-1, D)
    index_np = np.asarray(index_np).astype(np.int64, copy=False).ravel()
    n_edges = input_np.shape[0]

    order, seg_base, nseg, e_start, ne = pack_chunks(index_np, n_segments)
    n_chunks = len(seg_base)
    per_core = -(-n_chunks // N_CORES)
    per_core = -(-per_core // CPS) * CPS          # whole SGs
    total_chunks = per_core * N_CORES
    n_sg = per_core // CPS

    edge_chunk = np.repeat(np.arange(n_chunks), ne)
    within = np.arange(n_edges) - np.repeat(e_start, ne)
    slot = edge_chunk * EDGES_PER_CHUNK + within

    idx_sorted = index_np[order]
    local_row = (idx_sorted - seg_base[edge_chunk]).astype(NP_F16)

    if os.environ.get("NO_EF", "0") == "1":
        q = input_np[order].astype(NP_F8)
    else:
        q = encode_fp8_ef(input_np[order], idx_sorted, n_segments)

    total_slots = total_chunks * EDGES_PER_CHUNK
    X_all = np.zeros((total_slots, D), dtype=NP_F8)
    X_all[slot] = q
    L_all = np.zeros(total_slots, dtype=NP_F16)
    L_all[slot] = local_row

    n_tiles_core = per_core * TPC
    iota = np.broadcast_to(
        np.arange(G, dtype=NP_F16)[:, None], (G, SGT)
    ).reshape(1, G * SGT)
    iota = np.broadcast_to(iota, (P, G * SGT)).copy()

    in_maps = []
    for c in range(N_CORES):
        lo_s = c * per_core * EDGES_PER_CHUNK
        hi_s = lo_s + per_core * EDGES_PER_CHUNK
        xt = X_all[lo_s:hi_s].reshape(n_tiles_core, P, D)
        xc = xt.transpose(1, 0, 2).reshape(P, n_tiles_core * D)
        lc = L_all[lo_s:hi_s].reshape(n_tiles_core, P).transpose(1, 0)
        in_maps.append(
            {
                "x": np.ascontiguousarray(xc),
                "l": np.ascontiguousarray(lc),
                "iota": iota,
            }
        )

    n_blocks = -(-n_sg // BANDS)

    def assemble(core_outs):
        # core out: [128, n_blocks*512] f16; SG s at band s%4, block s//4,
        # chunk slot cc cols cc*64..; rows = 32 local segs
        rows = np.empty((total_chunks * G, D), dtype=np.float32)
        for c, o in enumerate(core_outs):
            o = np.asarray(o, dtype=np.float32).reshape(
                4, G, n_blocks, BANDS, 2, D
            )
            # [band, g, blk, sub, slot, d]: sg = blk*4+sub,
            # chunk-in-sg = slot*4+band
            r = o.transpose(2, 3, 4, 0, 1, 5).reshape(
                n_blocks * BANDS, CPS * G, D
            )
            r = r[:n_sg].reshape(per_core * G, D)
            rows[c * per_core * G : (c + 1) * per_core * G] = r
        row_seg = np.full(total_chunks * G, -1, dtype=np.int64)
        for i in range(n_chunks):
            row_seg[i * G : i * G + nseg[i]] = np.arange(
                seg_base[i], seg_base[i] + nseg[i]
            )
        valid = row_seg >= 0
        out = np.zeros((n_segments, D), dtype=np.float32)
        targets = row_seg[valid]
        vals = rows[valid]
        if len(np.unique(targets)) == len(targets):
            out[targets] = vals
        else:
            np.add.at(out, targets, vals)
        return out

    return per_core, in_maps, assemble


# --------------------------------------------------------------------------
# device kernel
# --------------------------------------------------------------------------

def build_strips(n_sg):
    """(sg_start, n_sgs) per x DMA strip: small head, then STRIP_SGS."""
    strips = []
    s = 0
    if HEAD_SGS and n_sg > HEAD_SGS:
        strips.append((0, HEAD_SGS))
        s = HEAD_SGS
    while s < n_sg:
        take = min(STRIP_SGS, n_sg - s)
        strips.append((s, take))
        s += take
    return strips


def build_bass(per_core: int):
    nc = bacc.Bacc(
        "TRN2", target_bir_lowering=False, debug=False, num_devices=N_CORES
    )
    assert per_core % CPS == 0
    n_tiles = per_core * TPC
    n_sg = per_core // CPS
    n_blocks = -(-n_sg // BANDS)

    X = nc.dram_tensor("x", [P, n_tiles * D], F8, kind="ExternalInput")
    L = nc.dram_tensor("l", [P, n_tiles], F16, kind="ExternalInput")
    IOTA = nc.dram_tensor("iota", [P, G * SGT], F16, kind="ExternalInput")
    OUT = nc.dram_tensor(
        "out", [P, n_blocks * CPS * D], F16, kind="ExternalOutput"
    )

    strips = build_strips(n_sg)
    sg_strip = np.zeros(n_sg, dtype=np.int64)     # strip index of each SG
    strip_cum = []                                 # SGs completed through strip k
    for k, (s0, nsg_k) in enumerate(strips):
        sg_strip[s0 : s0 + nsg_k] = k
        strip_cum.append(s0 + nsg_k)
    max_strip_w = max(nsg_k for _, nsg_k in strips) * SGT * D

    with TileContext(nc) as tc:
        with (
            tc.tile_pool(name="const", bufs=1) as cpool,
            tc.tile_pool(name="xin", bufs=1) as xpool,
            tc.tile_pool(name="oh", bufs=1) as ohpool,
            tc.tile_pool(name="acc", bufs=1, space="PSUM") as ppool,
            tc.tile_pool(name="outp", bufs=1) as opool,
        ):
            iota_t = cpool.tile([P, G * SGT], F16)
            nc.sync.dma_start(out=iota_t[:], in_=IOTA[:, :])
            l_t = cpool.tile([P, n_tiles], F16)
            nc.sync.dma_start(out=l_t[:], in_=L[:, :])

            xbufs = [xpool.tile([P, max_strip_w], F8, tag=f"x{i}", name=f"x{i}")
                     for i in range(XBUFS)]
            ohbufs = [ohpool.tile([P, G * SGT], F16, tag=f"oh{i}", name=f"oh{i}")
                      for i in range(OHBUFS)]
            psbufs = [ppool.tile([P, 2 * D], F32, tag=f"ps{i}", name=f"ps{i}")
                      for i in range(PSBUFS)]
            obufs = [opool.tile([P, CPS * D], F16, tag=f"ob{i}", name=f"ob{i}")
                     for i in range(OBUFS)]

            sem_x = nc.alloc_semaphore("sx")
            sem_oh = nc.alloc_semaphore("soh")
            sem_pe = nc.alloc_semaphore("spe")
            sem_fl = nc.alloc_semaphore("sfl")
            sem_st = nc.alloc_semaphore("sst")

            with tc.tile_critical():
                # ---- x strip DMAs (sync HWDGE, in order) ----
                for k, (s0, nsg_k) in enumerate(strips):
                    w = nsg_k * SGT * D
                    if k >= XBUFS:
                        # buffer reuse: all SGs of strip k-XBUFS consumed
                        nc.sync.wait_ge(sem_pe, strip_cum[k - XBUFS])
                    nc.sync.dma_start(
                        out=xbufs[k % XBUFS][:, :w],
                        in_=X[:, s0 * SGT * D : s0 * SGT * D + w],
                    ).then_inc(sem_x, 16)

                # ---- DVE one-hot per SG ----
                for s in range(n_sg):
                    oh = ohbufs[s % OHBUFS]
                    lb = (
                        l_t[:, s * SGT : (s + 1) * SGT]
                        .unsqueeze(1)
                        .broadcast_to([P, G, SGT])
                    )
                    if s >= OHBUFS:
                        nc.vector.wait_ge(sem_pe, s - OHBUFS + 1)
                    nc.vector.tensor_tensor(
                        oh[:].rearrange("p (g t) -> p g t", g=G, t=SGT),
                        iota_t[:].rearrange("p (g t) -> p g t", g=G, t=SGT),
                        lb,
                        AluOpType.is_equal,
                    ).then_inc(sem_oh)

                # ---- PE: 64 (LDW+MM) per SG ----
                for s in range(n_sg):
                    k = int(sg_strip[s])
                    s0 = strips[k][0]
                    xb = xbufs[k % XBUFS]
                    ohv = ohbufs[s % OHBUFS][:].rearrange(
                        "p (g t) -> p g t", g=G, t=SGT
                    )
                    ps = psbufs[s % PSBUFS]
                    nc.tensor.wait_ge(sem_oh, s + 1)
                    nc.tensor.wait_ge(sem_x, 16 * (k + 1))
                    if s >= PSBUFS:
                        nc.tensor.wait_ge(sem_fl, s - PSBUFS + 1)
                    # col-group alternation: consecutive MMs hit different
                    # PE column strips so their LDWEIGHTS overlap
                    for slot in range(2):
                        for t in range(TPC):
                            for band in range(4):
                                cc = slot * 4 + band
                                t64 = cc * TPC + t
                                xoff = ((s - s0) * SGT + t64) * D
                                mm = nc.tensor.matmul(
                                    ps[band * G : (band + 1) * G,
                                       slot * D : (slot + 1) * D],
                                    lhsT=ohv[:, :, t64],
                                    rhs=xb[:, xoff : xoff + D],
                                    start=(t == 0),
                                    stop=(t == TPC - 1),
                                    tile_position=(0, band * G),
                                    skip_group_check=True,
                                )
                    mm.then_inc(sem_pe)

                # ---- ACT: flush PSUM -> out band; DMA per block ----
                for s in range(n_sg):
                    sub = s % BANDS
                    blk = s // BANDS
                    ob = obufs[blk % OBUFS]
                    nc.scalar.wait_ge(sem_pe, s + 1)
                    if sub == 0 and blk >= OBUFS:
                        nc.scalar.wait_ge(sem_st, 16 * (blk - OBUFS + 1))
                    nc.scalar.copy(
                        ob[:, sub * 2 * D : (sub + 1) * 2 * D],
                        psbufs[s % PSBUFS][:, :],
                    ).then_inc(sem_fl)
                    if sub == BANDS - 1 or s == n_sg - 1:
                        nc.scalar.dma_start(
                            out=OUT[:, blk * CPS * D : (blk + 1) * CPS * D],
                            in_=ob[:],
                        ).then_inc(sem_st, 16)

                # ---- final completion ----
                nc.sync.wait_ge(sem_st, 16 * n_blocks)
    nc.compile()
    return nc


# --------------------------------------------------------------------------
# entry point
# --------------------------------------------------------------------------

def _run(input_np, index_np, n_segments, trace=False, trace_kwargs=None):
    per_core, in_maps, assemble = build_device_arrays(
        input_np, index_np, n_segments
    )
    nc = build_bass(per_core)
    res = run_bass_kernel_spmd(
        nc,
        in_maps,
        core_ids=list(range(N_CORES)),
        trace=trace,
        **(trace_kwargs or {}),
    )
    outs = [np.asarray(r["out"]) for r in res.results]
    return assemble(outs), res


def kernel(input, index):
    out, _ = _run(np.asarray(input), np.asarray(index), 50000)
    return out


# revision 15
# speedup vs baseline: 1.0189x; 1.0189x over previous
"""Segment-sum (scatter-add) kernel for Trainium2, 8 NeuronCores.

out[n, :] = sum_{e : index[e] == n} input[e, :]   (N=50000 segments, d=64)

Host side (data movement / re-encoding only; every FLOP of the actual
reduction runs on device):
  1. argsort(index); greedily pack whole segments into chunks of
     <= 32 consecutive segment ids and <= 1024 edges (8 tiles x 128).
  2. Edge rows are re-encoded fp32 -> fp8e3 (E3M4) with per-segment
     error-feedback rounding (measured rel err 3.8e-3 vs the 2e-2
     gate); halves HBM traffic again vs an fp16 encoding.
  3. Chunks are split contiguously across 8 cores, partition-major
     tile layout so every DMA is a dense strip.

Device side (raw bass, no TileContext): "super-groups" (SG) of
8 chunks = 64 tiles; all engines run free with credit semaphores.
  - sync: one x DMA per 2-SG strip into a 4-SG-slot SBUF ring.
  - DVE: one is_equal per SG builds the one-hot in g-major layout
    oh[p, g*64+t] against a dense iota constant (2x_1P DVE mode).
  - PE: per tile LDWEIGHTS(32 col, stride 64) + MATMUL(N=64, fp8 rhs x
    fp16 lhsT) with 4-way column-group alternation (consecutive MMs
    hit different PE column strips -> LDWEIGHTS overlap; 14ns/MM).
    The whole PE stream runs in ONE hardware Fori loop over 4-SG
    static bodies so the loop body stays resident in IRAM --
    fully-unrolled streams stall ~4us every 16KiB of instruction
    fetch when the x DMA saturates HBM.
  - ACT: casts PSUM f32 -> f16 quarters of a [128, 512] out block
    (4 SGs), DMAs a block at a time.

Host finalization: pure scatter placement of per-chunk row blocks
(np.add.at only if a segment was ever split across chunks).
"""

import os
import sys
from contextlib import ExitStack

for _p in ("/opt/trn_rl_repo", "/opt/pypackages"):
    if _p not in sys.path:
        sys.path.append(_p)

import numpy as np
import ml_dtypes

import concourse.mybir as mybir
from concourse import bacc
from concourse.mybir import AluOpType
from concourse.bass_utils import run_bass_kernel_spmd

N_CORES = 8
P = 128               # partitions / contraction dim per tile
D = 64                # feature dim
G = 32                # segs per chunk / one-hot width
TPC = 8               # tiles per chunk
EDGES_PER_CHUNK = TPC * P   # 1024
CPS = 8               # chunks per SG
SGT = CPS * TPC       # tiles per SG = 64
U = 4                 # SG ring slots / PE loop body size
BANDS = 4             # SGs per out block

F32 = mybir.dt.float32
F16 = mybir.dt.float16
F8 = mybir.dt.float8e3
NP_F8 = ml_dtypes.float8_e3m4
NP_F16 = np.float16


# --------------------------------------------------------------------------
# host-side packing / re-encoding
# --------------------------------------------------------------------------

def pack_chunks(index: np.ndarray, n_segments: int):
    index = np.asarray(index).astype(np.int64, copy=False).ravel()
    order = np.argsort(index, kind="stable")
    counts = np.bincount(index, minlength=n_segments)

    seg_base, nsegs, edge_start, nedges = [], [], [], []
    s = 0
    epos = 0
    counts_list = counts.tolist()
    while s < n_segments:
        c = counts_list[s]
        if c > EDGES_PER_CHUNK:
            left = c
            while left > 0:
                take = min(left, EDGES_PER_CHUNK)
                seg_base.append(s); nsegs.append(1)
                edge_start.append(epos); nedges.append(take)
                epos += take
                left -= take
            s += 1
            continue
        base = s
        tot = 0
        ns = 0
        while (
            s < n_segments
            and ns < G
            and tot + counts_list[s] <= EDGES_PER_CHUNK
        ):
            tot += counts_list[s]
            ns += 1
            s += 1
        seg_base.append(base); nsegs.append(ns)
        edge_start.append(epos); nedges.append(tot)
        epos += tot
    return (
        order,
        np.array(seg_base, dtype=np.int64),
        np.array(nsegs, dtype=np.int64),
        np.array(edge_start, dtype=np.int64),
        np.array(nedges, dtype=np.int64),
    )


def encode_fp8_ef(xs: np.ndarray, ids: np.ndarray, n_segments: int):
    """Error-feedback fp8e3 rounding along each segment's edge chain."""
    counts = np.bincount(ids, minlength=n_segments)
    starts = np.concatenate([[0], np.cumsum(counts)[:-1]])
    pos = np.arange(len(ids)) - starts[ids]
    qs = np.empty(xs.shape, dtype=NP_F8)
    carry = np.zeros((n_segments, xs.shape[1]), dtype=np.float32)
    maxc = int(counts.max()) if len(counts) else 0
    for p_ in range(maxc):
        sel = np.nonzero(pos == p_)[0]
        if not len(sel):
            break
        segs = ids[sel]
        v = xs[sel] + carry[segs]
        qv = v.astype(NP_F8)
        carry[segs] = v - qv.astype(np.float32)
        qs[sel] = qv
    return qs


def build_device_arrays(input_np, index_np, n_segments):
    input_np = np.asarray(input_np, dtype=np.float32).reshape(-1, D)
    index_np = np.asarray(index_np).astype(np.int64, copy=False).ravel()
    n_edges = input_np.shape[0]

    order, seg_base, nseg, e_start, ne = pack_chunks(index_np, n_segments)
    n_chunks = len(seg_base)
    per_core = -(-n_chunks // N_CORES)
    per_core = -(-per_core // CPS) * CPS
    total_chunks = per_core * N_CORES
    n_sg = per_core // CPS

    edge_chunk = np.repeat(np.arange(n_chunks), ne)
    within = np.arange(n_edges) - np.repeat(e_start, ne)
    slot = edge_chunk * EDGES_PER_CHUNK + within

    idx_sorted = index_np[order]
    local_row = (idx_sorted - seg_base[edge_chunk]).astype(NP_F16)

    if os.environ.get("NO_EF", "0") == "1":
        q = input_np[order].astype(NP_F8)
    else:
        q = encode_fp8_ef(input_np[order], idx_sorted, n_segments)

    total_slots = total_chunks * EDGES_PER_CHUNK
    X_all = np.zeros((total_slots, D), dtype=NP_F8)
    X_all[slot] = q
    L_all = np.zeros(total_slots, dtype=NP_F16)
    L_all[slot] = local_row

    n_tiles_core = per_core * TPC
    iota = np.broadcast_to(
        np.arange(G, dtype=NP_F16)[:, None], (G, SGT)
    ).reshape(1, G * SGT)
    iota = np.broadcast_to(iota, (P, G * SGT)).copy()

    in_maps = []
    for c in range(N_CORES):
        lo_s = c * per_core * EDGES_PER_CHUNK
        hi_s = lo_s + per_core * EDGES_PER_CHUNK
        xt = X_all[lo_s:hi_s].reshape(n_tiles_core, P, D)
        xc = xt.transpose(1, 0, 2).reshape(P, n_tiles_core * D)
        lc = L_all[lo_s:hi_s].reshape(n_tiles_core, P).transpose(1, 0)
        in_maps.append(
            {
                "x": np.ascontiguousarray(xc),
                "l": np.ascontiguousarray(lc),
                "iota": iota,
            }
        )

    n_blocks = -(-n_sg // BANDS)

    def assemble(core_outs):
        rows = np.empty((total_chunks * G, D), dtype=np.float32)
        for c, o in enumerate(core_outs):
            o = np.asarray(o, dtype=np.float32).reshape(
                4, G, n_blocks, BANDS, 2, D
            )
            # [band, g, blk, sub, slot2, d]: sg = blk*4+sub,
            # chunk-in-sg = slot2*4+band
            r = o.transpose(2, 3, 4, 0, 1, 5).reshape(
                n_blocks * BANDS, CPS * G, D
            )
            r = r[:n_sg].reshape(per_core * G, D)
            rows[c * per_core * G : (c + 1) * per_core * G] = r
        row_seg = np.full(total_chunks * G, -1, dtype=np.int64)
        for i in range(n_chunks):
            row_seg[i * G : i * G + nseg[i]] = np.arange(
                seg_base[i], seg_base[i] + nseg[i]
            )
        valid = row_seg >= 0
        out = np.zeros((n_segments, D), dtype=np.float32)
        targets = row_seg[valid]
        vals = rows[valid]
        if len(np.unique(targets)) == len(targets):
            out[targets] = vals
        else:
            np.add.at(out, targets, vals)
        return out

    return per_core, in_maps, assemble


# --------------------------------------------------------------------------
# device kernel (raw bass)
# --------------------------------------------------------------------------

def build_bass(per_core: int):
    nc = bacc.Bacc(
        "TRN2", target_bir_lowering=False, debug=False, num_devices=N_CORES
    )
    assert per_core % CPS == 0
    n_tiles = per_core * TPC
    n_sg = per_core // CPS
    n_blocks = -(-n_sg // BANDS)
    n_loop = n_sg // U
    n_tail = n_sg % U
    n_strips = -(-n_sg // 2)

    X = nc.dram_tensor("x", [P, n_tiles * D], F8, kind="ExternalInput")
    L = nc.dram_tensor("l", [P, n_tiles], F16, kind="ExternalInput")
    IOTA = nc.dram_tensor("iota", [P, G * SGT], F16, kind="ExternalInput")
    OUT = nc.dram_tensor(
        "out", [P, n_blocks * CPS * D], F16, kind="ExternalOutput"
    )

    ctx = ExitStack()
    with ctx:
        iota_t = ctx.enter_context(nc.sbuf_tensor("iota_t", [P, G * SGT], F16))
        l_t = ctx.enter_context(nc.sbuf_tensor("l_t", [P, n_tiles], F16))
        xring = ctx.enter_context(nc.sbuf_tensor("xring", [P, U * SGT * D], F8))
        ohring = ctx.enter_context(
            nc.sbuf_tensor("ohring", [P, U * G * SGT], F16)
        )
        outb = ctx.enter_context(nc.sbuf_tensor("outb", [P, 2 * CPS * D], F16))
        psr = [
            ctx.enter_context(nc.psum_tensor(f"ps{j}", [P, 2 * D], F32))
            for j in range(U)
        ]

        s_pre = nc.alloc_semaphore("s_pre")   # preamble loads
        s_xa = nc.alloc_semaphore("s_xa")     # x strip arrival, even strips
        s_xb = nc.alloc_semaphore("s_xb")     # x strip arrival, odd strips
        s_xf = nc.alloc_semaphore("s_xf")     # x strips freed by PE (cum)
        s_ohr = nc.alloc_semaphore("s_ohr")   # oh SG ready (credit)
        s_ohf = nc.alloc_semaphore("s_ohf")   # oh slot free (credit)
        s_psf = nc.alloc_semaphore("s_psf")   # psum slot free (credit)
        s_ped = nc.alloc_semaphore("s_ped")   # PE SG done (cum)
        s_st = nc.alloc_semaphore("s_st")     # out block credit/completion

        # ---- preamble: constants + credit priming ----
        nc.sync.dma_start(out=iota_t[:], in_=IOTA[:, :]).then_inc(s_pre, 16)
        nc.sync.dma_start(out=l_t[:], in_=L[:, :]).then_inc(s_pre, 16)
        nc.sync.sem_inc(s_ohf, U)
        nc.sync.sem_inc(s_psf, U)
        nc.sync.wait_ge(s_pre, 32)
        nc.all_engine_barrier()

        # ---- sync: x strip DMAs (strip = 2 SGs -> ring slots (0,1)/(2,3)) --
        for k in range(n_strips):
            s0 = 2 * k
            nsg_k = min(2, n_sg - s0)
            w = nsg_k * SGT * D
            roff = (s0 % U) * SGT * D
            if k >= 2:
                nc.sync.wait_ge(s_xf, k)
            nc.sync.dma_start(
                out=xring[:, roff : roff + w],
                in_=X[:, s0 * SGT * D : s0 * SGT * D + w],
            ).then_inc(s_xa if k % 2 == 0 else s_xb, 16)

        # ---- DVE: one-hot per SG ----
        ohv = ohring[:].rearrange("p (u g t) -> p u g t", u=U, g=G, t=SGT)
        for s in range(n_sg):
            j = s % U
            nc.vector.wait_ge(s_ohf, s + 2)
            nc.vector.tensor_tensor(
                ohv[:, j],
                iota_t[:].rearrange("p (g t) -> p g t", g=G, t=SGT),
                l_t[:, s * SGT : (s + 1) * SGT]
                .unsqueeze(1)
                .broadcast_to([P, G, SGT]),
                AluOpType.is_equal,
            ).then_inc(s_ohr, 1)

        # ---- PE: hardware loop over U-SG static bodies ----
        r_xthr = nc.tensor.alloc_register("r_xthr")
        r_oh = nc.tensor.alloc_register("r_oh")
        r_ps = nc.tensor.alloc_register("r_ps")
        nc.tensor.reg_mov(r_xthr, 16)
        nc.tensor.reg_mov(r_oh, 1)
        nc.tensor.reg_mov(r_ps, 1)

        def pe_sg(j, release_strip_prev, release_oh_prev=True):
            # slot j even -> new strip: wait its parity arrival count
            if j % 2 == 0:
                sp = s_xa if (j // 2) % 2 == 0 else s_xb
                nc.tensor.wait_ge(sp, r_xthr)
            nc.tensor.wait_ge(s_ohr, r_oh)
            nc.tensor.reg_add(r_oh, r_oh, 1)
            nc.tensor.wait_ge(s_psf, r_ps)
            nc.tensor.reg_add(r_ps, r_ps, 1)
            ps = psr[j]
            n_mm = 0
            for s2 in range(2):
                for t in range(TPC):
                    for band in range(4):
                        cc = s2 * 4 + band
                        t64 = cc * TPC + t
                        mm = nc.tensor.matmul(
                            ps[band * G : (band + 1) * G,
                               s2 * D : (s2 + 1) * D],
                            lhsT=ohv[:, j, :, t64],
                            rhs=xring[:, (j * SGT + t64) * D
                                      : (j * SGT + t64) * D + D],
                            start=(t == 0),
                            stop=(t == TPC - 1),
                            tile_position=(0, band * G),
                            skip_group_check=True,
                        )
                        n_mm += 1
                        if n_mm == 1 and release_oh_prev:
                            # previous ring slot's oh is fully consumed
                            # once this MM (strictly later in PE FIFO
                            # order) completes
                            mm.then_inc(s_ohf, 1)
                        elif n_mm == 2 and release_strip_prev:
                            # the strip ending at the previous slot is
                            # fully consumed once this MM completes
                            mm.then_inc(s_xf, 1)
                        elif n_mm == 64:
                            mm.then_inc(s_ped, 1)

        # releases are unconditional: the first SG's "release" of its
        # non-existent predecessors contributes one spurious +1 to s_ohf
        # and s_xf each, absorbed in the consumer thresholds above
        if n_loop:
            with nc.tensor.Fori(0, n_loop):
                for j in range(U):
                    pe_sg(j, release_strip_prev=(j % 2 == 0))
                nc.tensor.reg_add(r_xthr, r_xthr, 16)
        for j in range(n_tail):
            pe_sg(j, release_strip_prev=(j % 2 == 0))

        # ---- ACT: flush PSUM -> out block quarters; DMA per block ----
        for s in range(n_sg):
            sub = s % BANDS
            blk = s // BANDS
            bo = (blk % 2) * CPS * D
            nc.scalar.wait_ge(s_ped, s + 1)
            if sub == 0 and blk >= 2:
                nc.scalar.wait_ge(s_st, 16 * (blk - 1))
            nc.scalar.copy(
                outb[:, bo + sub * 2 * D : bo + (sub + 1) * 2 * D],
                psr[s % U][:, :],
            ).then_inc(s_psf, 1)
            if sub == BANDS - 1 or s == n_sg - 1:
                # the block's copies must have fully drained before the
                # HWDGE doorbell lets the SDMA engines read outb
                nc.scalar.wait_ge(s_psf, U + s + 1)
                nc.scalar.dma_start(
                    out=OUT[:, blk * CPS * D : (blk + 1) * CPS * D],
                    in_=outb[:, bo : bo + CPS * D],
                ).then_inc(s_st, 16)
        nc.scalar.wait_ge(s_st, 16 * n_blocks)

        nc.all_engine_barrier()
    nc.compile()
    return nc


# --------------------------------------------------------------------------
# entry point
# --------------------------------------------------------------------------

def _run(input_np, index_np, n_segments, trace=False, trace_kwargs=None):
    per_core, in_maps, assemble = build_device_arrays(
        input_np, index_np, n_segments
    )
    nc = build_bass(per_core)
    res = run_bass_kernel_spmd(
        nc,
        in_maps,
        core_ids=list(range(N_CORES)),
        trace=trace,
        **(trace_kwargs or {}),
    )
    outs = [np.asarray(r["out"]) for r in res.results]
    return assemble(outs), res


def kernel(input, index):
    out, _ = _run(np.asarray(input), np.asarray(index), 50000)
    return out


# revision 20
# speedup vs baseline: 1.2162x; 1.1937x over previous
"""Segment-sum (scatter-add) kernel for Trainium2, 8 NeuronCores.

out[n, :] = sum_{e : index[e] == n} input[e, :]   (N=50000 segments, d=64)

Host side (data movement / re-encoding only; every FLOP of the actual
reduction runs on device):
  1. argsort(index); greedily pack whole segments into chunks of
     <= 32 consecutive segment ids and <= 1024 edges (8 tiles x 128).
  2. Edge rows are re-encoded fp32 -> fp8e3 (E3M4) with per-segment
     error-feedback rounding (measured rel err 3.8e-3 vs the 2e-2
     gate); halves HBM traffic again vs an fp16 encoding.
  3. Chunks are split contiguously across 8 cores, partition-major
     tile layout so every DMA is a dense strip.

Device side (raw bass, no TileContext): "super-groups" (SG) of
8 chunks = 64 tiles; all engines run free with credit semaphores.
  - sync: one x DMA per 2-SG strip into a 4-SG-slot SBUF ring.
  - DVE: one is_equal per SG builds the one-hot in g-major layout
    oh[p, g*64+t] against a dense iota constant (2x_1P DVE mode).
  - PE: per tile LDWEIGHTS(32 col, stride 64) + MATMUL(N=64, fp8 rhs x
    fp16 lhsT) with 4-way column-group alternation (consecutive MMs
    hit different PE column strips -> LDWEIGHTS overlap; 14ns/MM).
    The whole PE stream runs in ONE hardware Fori loop over 4-SG
    static bodies so the loop body stays resident in IRAM --
    fully-unrolled streams stall ~4us every 16KiB of instruction
    fetch when the x DMA saturates HBM.
  - ACT: casts PSUM f32 -> f16 quarters of a [128, 512] out block
    (4 SGs), DMAs a block at a time.

Host finalization: pure scatter placement of per-chunk row blocks
(np.add.at only if a segment was ever split across chunks).
"""

import os
import sys
from contextlib import ExitStack

for _p in ("/opt/trn_rl_repo", "/opt/pypackages"):
    if _p not in sys.path:
        sys.path.append(_p)

import numpy as np
import ml_dtypes

import concourse.mybir as mybir
from concourse import bacc
from concourse.mybir import AluOpType
from concourse.bass_utils import run_bass_kernel_spmd

N_CORES = 8
P = 128               # partitions / contraction dim per tile
D = 64                # feature dim
G = 32                # segs per chunk / one-hot width
TPC = 8               # tiles per chunk
EDGES_PER_CHUNK = TPC * P   # 1024
CPS = 8               # chunks per SG
SGT = CPS * TPC       # tiles per SG = 64
U = 6                 # SG ring slots / PE loop body size
BANDS = 4             # SGs per out block

F32 = mybir.dt.float32
F16 = mybir.dt.float16
F8 = mybir.dt.float8e3
NP_F8 = ml_dtypes.float8_e3m4
NP_F16 = np.float16


# --------------------------------------------------------------------------
# host-side packing / re-encoding
# --------------------------------------------------------------------------

def pack_chunks(index: np.ndarray, n_segments: int):
    index = np.asarray(index).astype(np.int64, copy=False).ravel()
    order = np.argsort(index, kind="stable")
    counts = np.bincount(index, minlength=n_segments)

    seg_base, nsegs, edge_start, nedges = [], [], [], []
    s = 0
    epos = 0
    counts_list = counts.tolist()
    while s < n_segments:
        c = counts_list[s]
        if c > EDGES_PER_CHUNK:
            left = c
            while left > 0:
                take = min(left, EDGES_PER_CHUNK)
                seg_base.append(s); nsegs.append(1)
                edge_start.append(epos); nedges.append(take)
                epos += take
                left -= take
            s += 1
            continue
        base = s
        tot = 0
        ns = 0
        while (
            s < n_segments
            and ns < G
            and tot + counts_list[s] <= EDGES_PER_CHUNK
        ):
            tot += counts_list[s]
            ns += 1
            s += 1
        seg_base.append(base); nsegs.append(ns)
        edge_start.append(epos); nedges.append(tot)
        epos += tot
    return (
        order,
        np.array(seg_base, dtype=np.int64),
        np.array(nsegs, dtype=np.int64),
        np.array(edge_start, dtype=np.int64),
        np.array(nedges, dtype=np.int64),
    )


def encode_fp8_ef(xs: np.ndarray, ids: np.ndarray, n_segments: int):
    """Error-feedback fp8e3 rounding along each segment's edge chain."""
    counts = np.bincount(ids, minlength=n_segments)
    starts = np.concatenate([[0], np.cumsum(counts)[:-1]])
    pos = np.arange(len(ids)) - starts[ids]
    qs = np.empty(xs.shape, dtype=NP_F8)
    carry = np.zeros((n_segments, xs.shape[1]), dtype=np.float32)
    maxc = int(counts.max()) if len(counts) else 0
    for p_ in range(maxc):
        sel = np.nonzero(pos == p_)[0]
        if not len(sel):
            break
        segs = ids[sel]
        v = xs[sel] + carry[segs]
        qv = v.astype(NP_F8)
        carry[segs] = v - qv.astype(np.float32)
        qs[sel] = qv
    return qs


def build_device_arrays(input_np, index_np, n_segments):
    input_np = np.asarray(input_np, dtype=np.float32).reshape(-1, D)
    index_np = np.asarray(index_np).astype(np.int64, copy=False).ravel()
    n_edges = input_np.shape[0]

    order, seg_base, nseg, e_start, ne = pack_chunks(index_np, n_segments)
    n_chunks = len(seg_base)
    per_core = -(-n_chunks // N_CORES)
    per_core = -(-per_core // CPS) * CPS
    total_chunks = per_core * N_CORES
    n_sg = per_core // CPS

    edge_chunk = np.repeat(np.arange(n_chunks), ne)
    within = np.arange(n_edges) - np.repeat(e_start, ne)
    slot = edge_chunk * EDGES_PER_CHUNK + within

    idx_sorted = index_np[order]
    local_row = (idx_sorted - seg_base[edge_chunk]).astype(NP_F16)

    if os.environ.get("NO_EF", "0") == "1":
        q = input_np[order].astype(NP_F8)
    else:
        q = encode_fp8_ef(input_np[order], idx_sorted, n_segments)

    total_slots = total_chunks * EDGES_PER_CHUNK
    X_all = np.zeros((total_slots, D), dtype=NP_F8)
    X_all[slot] = q
    L_all = np.zeros(total_slots, dtype=NP_F16)
    L_all[slot] = local_row

    n_tiles_core = per_core * TPC
    iota = np.broadcast_to(
        np.arange(G, dtype=NP_F16)[:, None], (G, SGT)
    ).reshape(1, G * SGT)
    iota = np.broadcast_to(iota, (P, G * SGT)).copy()

    in_maps = []
    for c in range(N_CORES):
        lo_s = c * per_core * EDGES_PER_CHUNK
        hi_s = lo_s + per_core * EDGES_PER_CHUNK
        xt = X_all[lo_s:hi_s].reshape(n_tiles_core, P, D)
        xc = xt.transpose(1, 0, 2).reshape(P, n_tiles_core * D)
        lc = L_all[lo_s:hi_s].reshape(n_tiles_core, P).transpose(1, 0)
        in_maps.append(
            {
                "x": np.ascontiguousarray(xc),
                "l": np.ascontiguousarray(lc),
                "iota": iota,
            }
        )

    n_blocks = -(-n_sg // BANDS)

    def assemble(core_outs):
        rows = np.empty((total_chunks * G, D), dtype=np.float32)
        for c, o in enumerate(core_outs):
            o = np.asarray(o, dtype=np.float32).reshape(
                4, G, n_blocks, BANDS, 2, D
            )
            # [band, g, blk, sub, slot2, d]: sg = blk*4+sub,
            # chunk-in-sg = slot2*4+band
            r = o.transpose(2, 3, 4, 0, 1, 5).reshape(
                n_blocks * BANDS, CPS * G, D
            )
            r = r[:n_sg].reshape(per_core * G, D)
            rows[c * per_core * G : (c + 1) * per_core * G] = r
        row_seg = np.full(total_chunks * G, -1, dtype=np.int64)
        for i in range(n_chunks):
            row_seg[i * G : i * G + nseg[i]] = np.arange(
                seg_base[i], seg_base[i] + nseg[i]
            )
        valid = row_seg >= 0
        out = np.zeros((n_segments, D), dtype=np.float32)
        targets = row_seg[valid]
        vals = rows[valid]
        if len(np.unique(targets)) == len(targets):
            out[targets] = vals
        else:
            np.add.at(out, targets, vals)
        return out

    return per_core, in_maps, assemble


# --------------------------------------------------------------------------
# device kernel (raw bass)
# --------------------------------------------------------------------------

def build_bass(per_core: int):
    nc = bacc.Bacc(
        "TRN2", target_bir_lowering=False, debug=False, num_devices=N_CORES
    )
    assert per_core % CPS == 0
    n_tiles = per_core * TPC
    n_sg = per_core // CPS
    n_blocks = -(-n_sg // BANDS)
    n_loop = n_sg // U
    n_tail = n_sg % U
    n_strips = -(-n_sg // 2)

    X = nc.dram_tensor("x", [P, n_tiles * D], F8, kind="ExternalInput")
    L = nc.dram_tensor("l", [P, n_tiles], F16, kind="ExternalInput")
    IOTA = nc.dram_tensor("iota", [P, G * SGT], F16, kind="ExternalInput")
    OUT = nc.dram_tensor(
        "out", [P, n_blocks * CPS * D], F16, kind="ExternalOutput"
    )

    ctx = ExitStack()
    with ctx:
        iota_t = ctx.enter_context(nc.sbuf_tensor("iota_t", [P, G * SGT], F16))
        l_t = ctx.enter_context(nc.sbuf_tensor("l_t", [P, n_tiles], F16))
        xring = ctx.enter_context(nc.sbuf_tensor("xring", [P, U * SGT * D], F8))
        ohring = ctx.enter_context(
            nc.sbuf_tensor("ohring", [P, U * G * SGT], F16)
        )
        outb = ctx.enter_context(nc.sbuf_tensor("outb", [P, 2 * CPS * D], F16))
        psr = [
            ctx.enter_context(nc.psum_tensor(f"ps{j}", [P, 2 * D], F32))
            for j in range(U)
        ]

        s_pre = nc.alloc_semaphore("s_pre")   # preamble loads
        s_x = [nc.alloc_semaphore(f"s_x{r}") for r in range(U // 2)]
        s_xf = nc.alloc_semaphore("s_xf")     # x strips freed by PE (cum)
        s_ohr = nc.alloc_semaphore("s_ohr")   # oh SG ready (credit)
        s_ohf = nc.alloc_semaphore("s_ohf")   # oh slot free (credit)
        s_psf = nc.alloc_semaphore("s_psf")   # psum slot free (credit)
        s_ped = nc.alloc_semaphore("s_ped")   # PE SG done (cum)
        s_st = nc.alloc_semaphore("s_st")     # out block credit/completion

        # ---- preamble: constants on the idle ACT queue, credits primed ----
        nc.scalar.dma_start(out=iota_t[:], in_=IOTA[:, :]).then_inc(s_pre, 16)
        nc.scalar.dma_start(out=l_t[:], in_=L[:, :]).then_inc(s_pre, 16)

        # ---- sync: x strip DMAs (strip = 2 SGs -> one of U//2 slot pairs) --
        for k in range(n_strips):
            s0 = 2 * k
            nsg_k = min(2, n_sg - s0)
            w = nsg_k * SGT * D
            roff = (s0 % U) * SGT * D
            if k >= U // 2:
                # ring slots of strip k-U//2 freed at strip k-U//2+1's
                # first-SG mm#2 (release count incl. the spurious first)
                nc.sync.wait_ge(s_xf, k - U // 2 + 2)
            nc.sync.dma_start(
                out=xring[:, roff : roff + w],
                in_=X[:, s0 * SGT * D : s0 * SGT * D + w],
            ).then_inc(s_x[k % (U // 2)], 16)

        # ---- DVE: one-hot per SG ----
        ohv = ohring[:].rearrange("p (u g t) -> p u g t", u=U, g=G, t=SGT)
        for s in range(n_sg):
            j = s % U
            if s == 0:
                nc.vector.wait_ge(s_pre, 32)
            if s >= U:
                # slot free: PE's release count (incl. the spurious first)
                nc.vector.wait_ge(s_ohf, s - U + 2)
            nc.vector.tensor_tensor(
                ohv[:, j],
                iota_t[:].rearrange("p (g t) -> p g t", g=G, t=SGT),
                l_t[:, s * SGT : (s + 1) * SGT]
                .unsqueeze(1)
                .broadcast_to([P, G, SGT]),
                AluOpType.is_equal,
            ).then_inc(s_ohr, 1)

        # ---- PE: peeled first round + hardware loop over U-SG bodies ----
        r_xthr = nc.tensor.alloc_register("r_xthr")
        r_oh = nc.tensor.alloc_register("r_oh")
        r_ps = nc.tensor.alloc_register("r_ps")
        nc.tensor.reg_mov(r_xthr, 32)
        nc.tensor.reg_mov(r_oh, U + 1)
        nc.tensor.reg_mov(r_ps, 1)

        def pe_sg(j, release_strip_prev, peel_s=None, release_oh_prev=True):
            # slot j even -> new strip: wait its slot-pair arrival count
            if peel_s is None:
                if j % 2 == 0:
                    nc.tensor.wait_ge(s_x[j // 2], r_xthr)
                nc.tensor.wait_ge(s_ohr, r_oh)
                nc.tensor.reg_add(r_oh, r_oh, 1)
                nc.tensor.wait_ge(s_psf, r_ps)
                nc.tensor.reg_add(r_ps, r_ps, 1)
            else:
                # first U SGs: literal thresholds, psum trivially free
                if j % 2 == 0:
                    nc.tensor.wait_ge(s_x[j // 2], 16)
                nc.tensor.wait_ge(s_ohr, peel_s + 1)
            ps = psr[j]
            n_mm = 0
            for s2 in range(2):
                for t in range(TPC):
                    for band in range(4):
                        cc = s2 * 4 + band
                        t64 = cc * TPC + t
                        mm = nc.tensor.matmul(
                            ps[band * G : (band + 1) * G,
                               s2 * D : (s2 + 1) * D],
                            lhsT=ohv[:, j, :, t64],
                            rhs=xring[:, (j * SGT + t64) * D
                                      : (j * SGT + t64) * D + D],
                            start=(t == 0),
                            stop=(t == TPC - 1),
                            tile_position=(0, band * G),
                            skip_group_check=True,
                        )
                        n_mm += 1
                        if n_mm == 1 and release_oh_prev:
                            # previous ring slot's oh is fully consumed
                            # once this MM (strictly later in PE FIFO
                            # order) completes
                            mm.then_inc(s_ohf, 1)
                        elif n_mm == 2 and release_strip_prev:
                            # the strip ending at the previous slot is
                            # fully consumed once this MM completes
                            mm.then_inc(s_xf, 1)
                        elif n_mm == 64:
                            mm.then_inc(s_ped, 1)

        # releases are unconditional: the first SG's "release" of its
        # non-existent predecessors contributes one spurious +1 to s_ohf
        # and s_xf each, absorbed in the consumer thresholds above
        n_peel = min(U, n_sg)
        for j in range(n_peel):
            pe_sg(j, release_strip_prev=(j % 2 == 0), peel_s=j)
        n_loop2 = (n_sg - n_peel) // U
        n_tail2 = (n_sg - n_peel) % U
        if n_loop2:
            with nc.tensor.Fori(0, n_loop2):
                for j in range(U):
                    pe_sg(j, release_strip_prev=(j % 2 == 0))
                nc.tensor.reg_add(r_xthr, r_xthr, 16)
        for j in range(n_tail2):
            pe_sg(j, release_strip_prev=(j % 2 == 0))

        # ---- ACT: flush PSUM -> out block quarters; DMA per block ----
        for s in range(n_sg):
            sub = s % BANDS
            blk = s // BANDS
            bo = (blk % 2) * CPS * D
            nc.scalar.wait_ge(s_ped, s + 1)
            if sub == 0 and blk >= 2:
                nc.scalar.wait_ge(s_st, 16 * (blk - 1))
            nc.scalar.copy(
                outb[:, bo + sub * 2 * D : bo + (sub + 1) * 2 * D],
                psr[s % U][:, :],
            ).then_inc(s_psf, 1)
            if sub == BANDS - 1 or s == n_sg - 1:
                # the block's copies must have fully drained before the
                # HWDGE doorbell lets the SDMA engines read outb
                nc.scalar.wait_ge(s_psf, s + 1)
                nc.scalar.dma_start(
                    out=OUT[:, blk * CPS * D : (blk + 1) * CPS * D],
                    in_=outb[:, bo : bo + CPS * D],
                ).then_inc(s_st, 16)
        nc.scalar.wait_ge(s_st, 16 * n_blocks)
    nc.compile()
    return nc


# --------------------------------------------------------------------------
# entry point
# --------------------------------------------------------------------------

def _run(input_np, index_np, n_segments, trace=False, trace_kwargs=None):
    per_core, in_maps, assemble = build_device_arrays(
        input_np, index_np, n_segments
    )
    nc = build_bass(per_core)
    res = run_bass_kernel_spmd(
        nc,
        in_maps,
        core_ids=list(range(N_CORES)),
        trace=trace,
        **(trace_kwargs or {}),
    )
    outs = [np.asarray(r["out"]) for r in res.results]
    return assemble(outs), res


def kernel(input, index):
    out, _ = _run(np.asarray(input), np.asarray(index), 50000)
    return out


# revision 22
# speedup vs baseline: 1.2602x; 1.0361x over previous
"""Segment-sum (scatter-add) kernel for Trainium2, 8 NeuronCores.

out[n, :] = sum_{e : index[e] == n} input[e, :]   (N=50000 segments, d=64)

Host side (data movement / re-encoding only; every FLOP of the actual
reduction runs on device):
  1. argsort(index); greedily pack whole segments into chunks of
     <= 32 consecutive segment ids and <= 1024 edges (8 tiles x 128).
  2. Edge rows are re-encoded fp32 -> fp8e3 (E3M4) with per-segment
     error-feedback rounding (measured rel err 3.8e-3 vs the 2e-2
     gate); halves HBM traffic again vs an fp16 encoding.
  3. Chunks are split contiguously across 8 cores, partition-major
     tile layout so every DMA is a dense strip.

Device side (raw bass, no TileContext): "super-groups" (SG) of
8 chunks = 64 tiles; all engines run free with credit semaphores.
  - sync: one x DMA per 2-SG strip into a 4-SG-slot SBUF ring.
  - DVE: one is_equal per SG builds the one-hot in g-major layout
    oh[p, g*64+t] against a dense iota constant (2x_1P DVE mode).
  - PE: per tile LDWEIGHTS(32 col, stride 64) + MATMUL(N=64, fp8 rhs x
    fp16 lhsT) with 4-way column-group alternation (consecutive MMs
    hit different PE column strips -> LDWEIGHTS overlap; 14ns/MM).
    The whole PE stream runs in ONE hardware Fori loop over 4-SG
    static bodies so the loop body stays resident in IRAM --
    fully-unrolled streams stall ~4us every 16KiB of instruction
    fetch when the x DMA saturates HBM.
  - ACT: casts PSUM f32 -> f16 quarters of a [128, 512] out block
    (4 SGs), DMAs a block at a time.

Host finalization: pure scatter placement of per-chunk row blocks
(np.add.at only if a segment was ever split across chunks).
"""

import os
import sys
from contextlib import ExitStack

for _p in ("/opt/trn_rl_repo", "/opt/pypackages"):
    if _p not in sys.path:
        sys.path.append(_p)

import numpy as np
import ml_dtypes

import concourse.mybir as mybir
from concourse import bacc
from concourse.mybir import AluOpType
from concourse.bass_utils import run_bass_kernel_spmd

N_CORES = 8
P = 128               # partitions / contraction dim per tile
D = 64                # feature dim
G = 32                # segs per chunk / one-hot width
TPC = 8               # tiles per chunk
EDGES_PER_CHUNK = TPC * P   # 1024
CPS = 8               # chunks per SG
SGT = CPS * TPC       # tiles per SG = 64
U = 6                 # SG ring slots / PE loop body size
BANDS = 4             # SGs per out block

F32 = mybir.dt.float32
F16 = mybir.dt.float16
F8 = mybir.dt.float8e3
NP_F8 = ml_dtypes.float8_e3m4
NP_F16 = np.float16


# --------------------------------------------------------------------------
# host-side packing / re-encoding
# --------------------------------------------------------------------------

def pack_chunks(index: np.ndarray, n_segments: int):
    index = np.asarray(index).astype(np.int64, copy=False).ravel()
    order = np.argsort(index, kind="stable")
    counts = np.bincount(index, minlength=n_segments)

    seg_base, nsegs, edge_start, nedges = [], [], [], []
    s = 0
    epos = 0
    counts_list = counts.tolist()
    while s < n_segments:
        c = counts_list[s]
        if c > EDGES_PER_CHUNK:
            left = c
            while left > 0:
                take = min(left, EDGES_PER_CHUNK)
                seg_base.append(s); nsegs.append(1)
                edge_start.append(epos); nedges.append(take)
                epos += take
                left -= take
            s += 1
            continue
        base = s
        tot = 0
        ns = 0
        while (
            s < n_segments
            and ns < G
            and tot + counts_list[s] <= EDGES_PER_CHUNK
        ):
            tot += counts_list[s]
            ns += 1
            s += 1
        seg_base.append(base); nsegs.append(ns)
        edge_start.append(epos); nedges.append(tot)
        epos += tot
    return (
        order,
        np.array(seg_base, dtype=np.int64),
        np.array(nsegs, dtype=np.int64),
        np.array(edge_start, dtype=np.int64),
        np.array(nedges, dtype=np.int64),
    )


def encode_fp8_ef(xs: np.ndarray, ids: np.ndarray, n_segments: int):
    """Error-feedback fp8e3 rounding along each segment's edge chain."""
    counts = np.bincount(ids, minlength=n_segments)
    starts = np.concatenate([[0], np.cumsum(counts)[:-1]])
    pos = np.arange(len(ids)) - starts[ids]
    qs = np.empty(xs.shape, dtype=NP_F8)
    carry = np.zeros((n_segments, xs.shape[1]), dtype=np.float32)
    maxc = int(counts.max()) if len(counts) else 0
    for p_ in range(maxc):
        sel = np.nonzero(pos == p_)[0]
        if not len(sel):
            break
        segs = ids[sel]
        v = xs[sel] + carry[segs]
        qv = v.astype(NP_F8)
        carry[segs] = v - qv.astype(np.float32)
        qs[sel] = qv
    return qs


def build_device_arrays(input_np, index_np, n_segments):
    input_np = np.asarray(input_np, dtype=np.float32).reshape(-1, D)
    index_np = np.asarray(index_np).astype(np.int64, copy=False).ravel()
    n_edges = input_np.shape[0]

    order, seg_base, nseg, e_start, ne = pack_chunks(index_np, n_segments)
    n_chunks = len(seg_base)
    per_core = -(-n_chunks // N_CORES)
    per_core = -(-per_core // CPS) * CPS
    total_chunks = per_core * N_CORES
    n_sg = per_core // CPS

    edge_chunk = np.repeat(np.arange(n_chunks), ne)
    within = np.arange(n_edges) - np.repeat(e_start, ne)
    slot = edge_chunk * EDGES_PER_CHUNK + within

    idx_sorted = index_np[order]
    local_row = (idx_sorted - seg_base[edge_chunk]).astype(NP_F16)

    if os.environ.get("NO_EF", "0") == "1":
        q = input_np[order].astype(NP_F8)
    else:
        q = encode_fp8_ef(input_np[order], idx_sorted, n_segments)

    total_slots = total_chunks * EDGES_PER_CHUNK
    X_all = np.zeros((total_slots, D), dtype=NP_F8)
    X_all[slot] = q
    L_all = np.zeros(total_slots, dtype=NP_F16)
    L_all[slot] = local_row

    n_tiles_core = per_core * TPC
    iota = np.broadcast_to(
        np.arange(G, dtype=NP_F16)[:, None], (G, SGT)
    ).reshape(1, G * SGT)
    iota = np.broadcast_to(iota, (P, G * SGT)).copy()

    in_maps = []
    for c in range(N_CORES):
        lo_s = c * per_core * EDGES_PER_CHUNK
        hi_s = lo_s + per_core * EDGES_PER_CHUNK
        xt = X_all[lo_s:hi_s].reshape(n_tiles_core, P, D)
        xc = xt.transpose(1, 0, 2).reshape(P, n_tiles_core * D)
        lc = L_all[lo_s:hi_s].reshape(n_tiles_core, P).transpose(1, 0)
        in_maps.append(
            {
                "x": np.ascontiguousarray(xc),
                "l": np.ascontiguousarray(lc),
                "iota": iota,
            }
        )

    n_blocks = -(-n_sg // BANDS)

    def assemble(core_outs):
        rows = np.empty((total_chunks * G, D), dtype=np.float32)
        for c, o in enumerate(core_outs):
            o = np.asarray(o, dtype=np.float32).reshape(
                4, G, n_blocks, BANDS, 2, D
            )
            # [band, g, blk, sub, slot2, d]: sg = blk*4+sub,
            # chunk-in-sg = slot2*4+band
            r = o.transpose(2, 3, 4, 0, 1, 5).reshape(
                n_blocks * BANDS, CPS * G, D
            )
            r = r[:n_sg].reshape(per_core * G, D)
            rows[c * per_core * G : (c + 1) * per_core * G] = r
        row_seg = np.full(total_chunks * G, -1, dtype=np.int64)
        for i in range(n_chunks):
            row_seg[i * G : i * G + nseg[i]] = np.arange(
                seg_base[i], seg_base[i] + nseg[i]
            )
        valid = row_seg >= 0
        out = np.zeros((n_segments, D), dtype=np.float32)
        targets = row_seg[valid]
        vals = rows[valid]
        if len(np.unique(targets)) == len(targets):
            out[targets] = vals
        else:
            np.add.at(out, targets, vals)
        return out

    return per_core, in_maps, assemble


# --------------------------------------------------------------------------
# device kernel (raw bass)
# --------------------------------------------------------------------------

def build_bass(per_core: int):
    nc = bacc.Bacc(
        "TRN2", target_bir_lowering=False, debug=False, num_devices=N_CORES
    )
    assert per_core % CPS == 0
    n_tiles = per_core * TPC
    n_sg = per_core // CPS
    n_blocks = -(-n_sg // BANDS)
    n_loop = n_sg // U
    n_tail = n_sg % U
    n_strips = -(-n_sg // 2)

    X = nc.dram_tensor("x", [P, n_tiles * D], F8, kind="ExternalInput")
    L = nc.dram_tensor("l", [P, n_tiles], F16, kind="ExternalInput")
    IOTA = nc.dram_tensor("iota", [P, G * SGT], F16, kind="ExternalInput")
    OUT = nc.dram_tensor(
        "out", [P, n_blocks * CPS * D], F16, kind="ExternalOutput"
    )

    ctx = ExitStack()
    with ctx:
        iota_t = ctx.enter_context(nc.sbuf_tensor("iota_t", [P, G * SGT], F16))
        l_t = ctx.enter_context(nc.sbuf_tensor("l_t", [P, n_tiles], F16))
        xring = ctx.enter_context(nc.sbuf_tensor("xring", [P, U * SGT * D], F8))
        ohring = ctx.enter_context(
            nc.sbuf_tensor("ohring", [P, U * G * SGT], F16)
        )
        outb = ctx.enter_context(nc.sbuf_tensor("outb", [P, 2 * CPS * D], F16))
        psr = [
            ctx.enter_context(nc.psum_tensor(f"ps{j}", [P, 2 * D], F32))
            for j in range(U)
        ]

        s_pre = nc.alloc_semaphore("s_pre")   # preamble loads
        s_x = [nc.alloc_semaphore(f"s_x{r}") for r in range(U // 2)]
        s_xf = nc.alloc_semaphore("s_xf")     # x strips freed by PE (cum)
        s_ohr = nc.alloc_semaphore("s_ohr")   # oh SG ready (credit)
        s_ohf = nc.alloc_semaphore("s_ohf")   # oh slot free (credit)
        s_psf = nc.alloc_semaphore("s_psf")   # psum slot free (credit)
        s_ped = nc.alloc_semaphore("s_ped")   # PE SG done (cum)
        s_st = nc.alloc_semaphore("s_st")     # out block credit/completion

        # ---- preamble: constants first on the sync queue (DVE gates on
        # them; the x strips queue up right behind) ----
        nc.sync.dma_start(out=l_t[:], in_=L[:, :]).then_inc(s_pre, 16)
        nc.scalar.dma_start(out=iota_t[:], in_=IOTA[:, :]).then_inc(s_pre, 16)

        # ---- x strip DMAs (strip = 2 SGs -> one of U//2 slot pairs),
        # alternating between the sync HWDGE queue and the otherwise-idle
        # gpsimd SWDGE queue so two transfers stream concurrently ----
        for k in range(n_strips):
            s0 = 2 * k
            nsg_k = min(2, n_sg - s0)
            w = nsg_k * SGT * D
            roff = (s0 % U) * SGT * D
            # engine fixed per slot-pair so each arrival sem has one owner
            eng = nc.gpsimd if k % (U // 2) == 1 else nc.sync
            if k >= U // 2:
                # ring slots of strip k-U//2 freed at strip k-U//2+1's
                # first-SG mm#2 (release count incl. the spurious first)
                eng.wait_ge(s_xf, k - U // 2 + 2)
            eng.dma_start(
                out=xring[:, roff : roff + w],
                in_=X[:, s0 * SGT * D : s0 * SGT * D + w],
            ).then_inc(s_x[k % (U // 2)], 16)

        # ---- DVE: one-hot per SG ----
        ohv = ohring[:].rearrange("p (u g t) -> p u g t", u=U, g=G, t=SGT)
        for s in range(n_sg):
            j = s % U
            if s == 0:
                nc.vector.wait_ge(s_pre, 32)
            if s >= U:
                # slot free: PE's release count (incl. the spurious first)
                nc.vector.wait_ge(s_ohf, s - U + 2)
            nc.vector.tensor_tensor(
                ohv[:, j],
                iota_t[:].rearrange("p (g t) -> p g t", g=G, t=SGT),
                l_t[:, s * SGT : (s + 1) * SGT]
                .unsqueeze(1)
                .broadcast_to([P, G, SGT]),
                AluOpType.is_equal,
            ).then_inc(s_ohr, 1)

        # ---- PE: peeled first round + hardware loop over U-SG bodies ----
        r_xthr = nc.tensor.alloc_register("r_xthr")
        r_oh = nc.tensor.alloc_register("r_oh")
        r_ps = nc.tensor.alloc_register("r_ps")
        nc.tensor.reg_mov(r_xthr, 32)
        nc.tensor.reg_mov(r_oh, U + 1)
        nc.tensor.reg_mov(r_ps, 1)

        def pe_sg(j, release_strip_prev, peel_s=None, release_oh_prev=True):
            # slot j even -> new strip: wait its slot-pair arrival count
            if peel_s is None:
                if j % 2 == 0:
                    nc.tensor.wait_ge(s_x[j // 2], r_xthr)
                nc.tensor.wait_ge(s_ohr, r_oh)
                nc.tensor.reg_add(r_oh, r_oh, 1)
                nc.tensor.wait_ge(s_psf, r_ps)
                nc.tensor.reg_add(r_ps, r_ps, 1)
            else:
                # first U SGs: literal thresholds, psum trivially free
                if j % 2 == 0:
                    nc.tensor.wait_ge(s_x[j // 2], 16)
                nc.tensor.wait_ge(s_ohr, peel_s + 1)
            ps = psr[j]
            n_mm = 0
            for s2 in range(2):
                for t in range(TPC):
                    for band in range(4):
                        cc = s2 * 4 + band
                        t64 = cc * TPC + t
                        mm = nc.tensor.matmul(
                            ps[band * G : (band + 1) * G,
                               s2 * D : (s2 + 1) * D],
                            lhsT=ohv[:, j, :, t64],
                            rhs=xring[:, (j * SGT + t64) * D
                                      : (j * SGT + t64) * D + D],
                            start=(t == 0),
                            stop=(t == TPC - 1),
                            tile_position=(0, band * G),
                            skip_group_check=True,
                        )
                        n_mm += 1
                        if n_mm == 1 and release_oh_prev:
                            # previous ring slot's oh is fully consumed
                            # once this MM (strictly later in PE FIFO
                            # order) completes
                            mm.then_inc(s_ohf, 1)
                        elif n_mm == 2 and release_strip_prev:
                            # the strip ending at the previous slot is
                            # fully consumed once this MM completes
                            mm.then_inc(s_xf, 1)
                        elif n_mm == 64:
                            mm.then_inc(s_ped, 1)

        # releases are unconditional: the first SG's "release" of its
        # non-existent predecessors contributes one spurious +1 to s_ohf
        # and s_xf each, absorbed in the consumer thresholds above
        n_peel = min(U, n_sg)
        for j in range(n_peel):
            pe_sg(j, release_strip_prev=(j % 2 == 0), peel_s=j)
        n_loop2 = (n_sg - n_peel) // U
        n_tail2 = (n_sg - n_peel) % U
        if n_loop2:
            with nc.tensor.Fori(0, n_loop2):
                for j in range(U):
                    pe_sg(j, release_strip_prev=(j % 2 == 0))
                nc.tensor.reg_add(r_xthr, r_xthr, 16)
        for j in range(n_tail2):
            pe_sg(j, release_strip_prev=(j % 2 == 0))

        # ---- ACT: flush PSUM -> out block quarters; DMA per block ----
        for s in range(n_sg):
            sub = s % BANDS
            blk = s // BANDS
            bo = (blk % 2) * CPS * D
            nc.scalar.wait_ge(s_ped, s + 1)
            if sub == 0 and blk >= 2:
                nc.scalar.wait_ge(s_st, 16 * (blk - 1))
            nc.scalar.copy(
                outb[:, bo + sub * 2 * D : bo + (sub + 1) * 2 * D],
                psr[s % U][:, :],
            ).then_inc(s_psf, 1)
            if sub == BANDS - 1 or s == n_sg - 1:
                # the block's copies must have fully drained before the
                # HWDGE doorbell lets the SDMA engines read outb
                nc.scalar.wait_ge(s_psf, s + 1)
                nc.scalar.dma_start(
                    out=OUT[:, blk * CPS * D : (blk + 1) * CPS * D],
                    in_=outb[:, bo : bo + CPS * D],
                ).then_inc(s_st, 16)
        nc.scalar.wait_ge(s_st, 16 * n_blocks)
    nc.compile()
    return nc


# --------------------------------------------------------------------------
# entry point
# --------------------------------------------------------------------------

def _run(input_np, index_np, n_segments, trace=False, trace_kwargs=None):
    per_core, in_maps, assemble = build_device_arrays(
        input_np, index_np, n_segments
    )
    nc = build_bass(per_core)
    res = run_bass_kernel_spmd(
        nc,
        in_maps,
        core_ids=list(range(N_CORES)),
        trace=trace,
        **(trace_kwargs or {}),
    )
    outs = [np.asarray(r["out"]) for r in res.results]
    return assemble(outs), res


def kernel(input, index):
    out, _ = _run(np.asarray(input), np.asarray(index), 50000)
    return out


# revision 23
# speedup vs baseline: 1.3590x; 1.0784x over previous
"""Segment-sum (scatter-add) kernel for Trainium2, 8 NeuronCores.

out[n, :] = sum_{e : index[e] == n} input[e, :]   (N=50000 segments, d=64)

Host side (data movement / re-encoding only; every FLOP of the actual
reduction runs on device):
  1. argsort(index); greedily pack whole segments into chunks of
     <= 32 consecutive segment ids and <= 1024 edges (8 tiles x 128).
  2. Edge rows are re-encoded fp32 -> fp8e3 (E3M4) with per-segment
     error-feedback rounding (measured rel err 3.8e-3 vs the 2e-2
     gate); halves HBM traffic again vs an fp16 encoding.
  3. Chunks are split contiguously across 8 cores, partition-major
     tile layout so every DMA is a dense strip.

Device side (raw bass, no TileContext): "super-groups" (SG) of
8 chunks = 64 tiles; all engines run free with credit semaphores.
  - sync: one x DMA per 2-SG strip into a 4-SG-slot SBUF ring.
  - DVE: one is_equal per SG builds the one-hot in g-major layout
    oh[p, g*64+t] against a dense iota constant (2x_1P DVE mode).
  - PE: per tile LDWEIGHTS(32 col, stride 64) + MATMUL(N=64, fp8 rhs x
    fp16 lhsT) with 4-way column-group alternation (consecutive MMs
    hit different PE column strips -> LDWEIGHTS overlap; 14ns/MM).
    The whole PE stream runs in ONE hardware Fori loop over 4-SG
    static bodies so the loop body stays resident in IRAM --
    fully-unrolled streams stall ~4us every 16KiB of instruction
    fetch when the x DMA saturates HBM.
  - ACT: casts PSUM f32 -> f16 quarters of a [128, 512] out block
    (4 SGs), DMAs a block at a time.

Host finalization: pure scatter placement of per-chunk row blocks
(np.add.at only if a segment was ever split across chunks).
"""

import os
import sys
from contextlib import ExitStack

for _p in ("/opt/trn_rl_repo", "/opt/pypackages"):
    if _p not in sys.path:
        sys.path.append(_p)

import numpy as np
import ml_dtypes

import concourse.mybir as mybir
from concourse import bacc
from concourse.mybir import AluOpType
from concourse.bass_utils import run_bass_kernel_spmd

N_CORES = 8
P = 128               # partitions / contraction dim per tile
D = 64                # feature dim
G = 32                # segs per chunk / one-hot width
TPC = 8               # tiles per chunk
EDGES_PER_CHUNK = TPC * P   # 1024
CPS = 8               # chunks per SG
SGT = CPS * TPC       # tiles per SG = 64
U = 6                 # SG ring slots / PE loop body size
BANDS = 4             # SGs per out block

F32 = mybir.dt.float32
F16 = mybir.dt.float16
F8 = mybir.dt.float8e3
NP_F8 = ml_dtypes.float8_e3m4
NP_F16 = np.float16


# --------------------------------------------------------------------------
# host-side packing / re-encoding
# --------------------------------------------------------------------------

def pack_chunks(index: np.ndarray, n_segments: int):
    index = np.asarray(index).astype(np.int64, copy=False).ravel()
    order = np.argsort(index, kind="stable")
    counts = np.bincount(index, minlength=n_segments)

    seg_base, nsegs, edge_start, nedges = [], [], [], []
    s = 0
    epos = 0
    counts_list = counts.tolist()
    while s < n_segments:
        c = counts_list[s]
        if c > EDGES_PER_CHUNK:
            left = c
            while left > 0:
                take = min(left, EDGES_PER_CHUNK)
                seg_base.append(s); nsegs.append(1)
                edge_start.append(epos); nedges.append(take)
                epos += take
                left -= take
            s += 1
            continue
        base = s
        tot = 0
        ns = 0
        while (
            s < n_segments
            and ns < G
            and tot + counts_list[s] <= EDGES_PER_CHUNK
        ):
            tot += counts_list[s]
            ns += 1
            s += 1
        seg_base.append(base); nsegs.append(ns)
        edge_start.append(epos); nedges.append(tot)
        epos += tot
    return (
        order,
        np.array(seg_base, dtype=np.int64),
        np.array(nsegs, dtype=np.int64),
        np.array(edge_start, dtype=np.int64),
        np.array(nedges, dtype=np.int64),
    )


def encode_fp8_ef(xs: np.ndarray, ids: np.ndarray, n_segments: int):
    """Error-feedback fp8e3 rounding along each segment's edge chain."""
    counts = np.bincount(ids, minlength=n_segments)
    starts = np.concatenate([[0], np.cumsum(counts)[:-1]])
    pos = np.arange(len(ids)) - starts[ids]
    qs = np.empty(xs.shape, dtype=NP_F8)
    carry = np.zeros((n_segments, xs.shape[1]), dtype=np.float32)
    maxc = int(counts.max()) if len(counts) else 0
    for p_ in range(maxc):
        sel = np.nonzero(pos == p_)[0]
        if not len(sel):
            break
        segs = ids[sel]
        v = xs[sel] + carry[segs]
        qv = v.astype(NP_F8)
        carry[segs] = v - qv.astype(np.float32)
        qs[sel] = qv
    return qs


def build_device_arrays(input_np, index_np, n_segments):
    input_np = np.asarray(input_np, dtype=np.float32).reshape(-1, D)
    index_np = np.asarray(index_np).astype(np.int64, copy=False).ravel()
    n_edges = input_np.shape[0]

    order, seg_base, nseg, e_start, ne = pack_chunks(index_np, n_segments)
    n_chunks = len(seg_base)
    per_core = -(-n_chunks // N_CORES)
    per_core = -(-per_core // CPS) * CPS
    total_chunks = per_core * N_CORES
    n_sg = per_core // CPS

    edge_chunk = np.repeat(np.arange(n_chunks), ne)
    within = np.arange(n_edges) - np.repeat(e_start, ne)
    slot = edge_chunk * EDGES_PER_CHUNK + within

    idx_sorted = index_np[order]
    local_row = (idx_sorted - seg_base[edge_chunk]).astype(NP_F16)

    if os.environ.get("NO_EF", "0") == "1":
        q = input_np[order].astype(NP_F8)
    else:
        q = encode_fp8_ef(input_np[order], idx_sorted, n_segments)

    total_slots = total_chunks * EDGES_PER_CHUNK
    X_all = np.zeros((total_slots, D), dtype=NP_F8)
    X_all[slot] = q
    L_all = np.zeros(total_slots, dtype=NP_F16)
    L_all[slot] = local_row

    n_tiles_core = per_core * TPC
    iota = np.broadcast_to(
        np.arange(G, dtype=NP_F16)[:, None], (G, SGT)
    ).reshape(1, G * SGT)
    iota = np.broadcast_to(iota, (P, G * SGT)).copy()

    in_maps = []
    for c in range(N_CORES):
        lo_s = c * per_core * EDGES_PER_CHUNK
        hi_s = lo_s + per_core * EDGES_PER_CHUNK
        xt = X_all[lo_s:hi_s].reshape(n_tiles_core, P, D)
        xc = xt.transpose(1, 0, 2).reshape(P, n_tiles_core * D)
        lc = L_all[lo_s:hi_s].reshape(n_tiles_core, P).transpose(1, 0)
        in_maps.append(
            {
                "x": np.ascontiguousarray(xc),
                "l": np.ascontiguousarray(lc),
                "iota": iota,
            }
        )

    n_blocks = -(-n_sg // BANDS)

    def assemble(core_outs):
        rows = np.empty((total_chunks * G, D), dtype=np.float32)
        for c, o in enumerate(core_outs):
            o = np.asarray(o, dtype=np.float32).reshape(
                4, G, n_blocks, BANDS, 2, D
            )
            # [band, g, blk, sub, slot2, d]: sg = blk*4+sub,
            # chunk-in-sg = slot2*4+band
            r = o.transpose(2, 3, 4, 0, 1, 5).reshape(
                n_blocks * BANDS, CPS * G, D
            )
            r = r[:n_sg].reshape(per_core * G, D)
            rows[c * per_core * G : (c + 1) * per_core * G] = r
        row_seg = np.full(total_chunks * G, -1, dtype=np.int64)
        for i in range(n_chunks):
            row_seg[i * G : i * G + nseg[i]] = np.arange(
                seg_base[i], seg_base[i] + nseg[i]
            )
        valid = row_seg >= 0
        out = np.zeros((n_segments, D), dtype=np.float32)
        targets = row_seg[valid]
        vals = rows[valid]
        if len(np.unique(targets)) == len(targets):
            out[targets] = vals
        else:
            np.add.at(out, targets, vals)
        return out

    return per_core, in_maps, assemble


# --------------------------------------------------------------------------
# device kernel (raw bass)
# --------------------------------------------------------------------------

def build_bass(per_core: int):
    nc = bacc.Bacc(
        "TRN2", target_bir_lowering=False, debug=False, num_devices=N_CORES
    )
    assert per_core % CPS == 0
    n_tiles = per_core * TPC
    n_sg = per_core // CPS
    n_blocks = -(-n_sg // BANDS)
    n_loop = n_sg // U
    n_tail = n_sg % U
    n_strips = n_sg

    X = nc.dram_tensor("x", [P, n_tiles * D], F8, kind="ExternalInput")
    L = nc.dram_tensor("l", [P, n_tiles], F16, kind="ExternalInput")
    IOTA = nc.dram_tensor("iota", [P, G * SGT], F16, kind="ExternalInput")
    OUT = nc.dram_tensor(
        "out", [P, n_blocks * CPS * D], F16, kind="ExternalOutput"
    )

    ctx = ExitStack()
    with ctx:
        iota_t = ctx.enter_context(nc.sbuf_tensor("iota_t", [P, G * SGT], F16))
        l_t = ctx.enter_context(nc.sbuf_tensor("l_t", [P, n_tiles], F16))
        xring = ctx.enter_context(nc.sbuf_tensor("xring", [P, U * SGT * D], F8))
        ohring = ctx.enter_context(
            nc.sbuf_tensor("ohring", [P, U * G * SGT], F16)
        )
        outb = ctx.enter_context(nc.sbuf_tensor("outb", [P, 2 * CPS * D], F16))
        psr = [
            ctx.enter_context(nc.psum_tensor(f"ps{j}", [P, 2 * D], F32))
            for j in range(U)
        ]

        s_pre = nc.alloc_semaphore("s_pre")   # preamble loads
        s_x = [nc.alloc_semaphore(f"s_x{r}") for r in range(U)]
        s_xf = nc.alloc_semaphore("s_xf")     # x strips freed by PE (cum)
        s_ohr = nc.alloc_semaphore("s_ohr")   # oh SG ready (credit)
        s_ohf = nc.alloc_semaphore("s_ohf")   # oh slot free (credit)
        s_psf = nc.alloc_semaphore("s_psf")   # psum slot free (credit)
        s_ped = nc.alloc_semaphore("s_ped")   # PE SG done (cum)
        s_st = nc.alloc_semaphore("s_st")     # out block credit/completion

        # ---- preamble: constants first on the sync queue (DVE gates on
        # them; the x strips queue up right behind) ----
        nc.sync.dma_start(out=l_t[:], in_=L[:, :]).then_inc(s_pre, 16)
        nc.scalar.dma_start(out=iota_t[:], in_=IOTA[:, :]).then_inc(s_pre, 16)

        # ---- x strip DMAs (strip = 1 SG -> its own ring slot + sem),
        # alternating between the sync HWDGE queue and the otherwise-idle
        # gpsimd SWDGE queue so two transfer streams run concurrently ----
        for k in range(n_strips):
            w = SGT * D
            roff = (k % U) * SGT * D
            # engine fixed per slot so each arrival sem has one owner
            eng = nc.sync if k % 2 == 0 else nc.gpsimd
            if k >= U:
                # ring slot of strip k-U freed at SG k-U+1's mm#2
                # (release count incl. the spurious first)
                eng.wait_ge(s_xf, k - U + 2)
            eng.dma_start(
                out=xring[:, roff : roff + w],
                in_=X[:, k * SGT * D : k * SGT * D + w],
            ).then_inc(s_x[k % U], 16)

        # ---- DVE: one-hot per SG ----
        ohv = ohring[:].rearrange("p (u g t) -> p u g t", u=U, g=G, t=SGT)
        for s in range(n_sg):
            j = s % U
            if s == 0:
                nc.vector.wait_ge(s_pre, 32)
            if s >= U:
                # slot free: PE's release count (incl. the spurious first)
                nc.vector.wait_ge(s_ohf, s - U + 2)
            nc.vector.tensor_tensor(
                ohv[:, j],
                iota_t[:].rearrange("p (g t) -> p g t", g=G, t=SGT),
                l_t[:, s * SGT : (s + 1) * SGT]
                .unsqueeze(1)
                .broadcast_to([P, G, SGT]),
                AluOpType.is_equal,
            ).then_inc(s_ohr, 1)

        # ---- PE: peeled first round + hardware loop over U-SG bodies ----
        r_xthr = nc.tensor.alloc_register("r_xthr")
        r_oh = nc.tensor.alloc_register("r_oh")
        r_ps = nc.tensor.alloc_register("r_ps")
        nc.tensor.reg_mov(r_xthr, 32)
        nc.tensor.reg_mov(r_oh, U + 1)
        nc.tensor.reg_mov(r_ps, 1)

        def pe_sg(j, release_strip_prev, peel_s=None, release_oh_prev=True):
            if peel_s is None:
                nc.tensor.wait_ge(s_x[j], r_xthr)
                nc.tensor.wait_ge(s_ohr, r_oh)
                nc.tensor.reg_add(r_oh, r_oh, 1)
                nc.tensor.wait_ge(s_psf, r_ps)
                nc.tensor.reg_add(r_ps, r_ps, 1)
            else:
                # first U SGs: literal thresholds, psum trivially free
                nc.tensor.wait_ge(s_x[j], 16)
                nc.tensor.wait_ge(s_ohr, peel_s + 1)
            ps = psr[j]
            n_mm = 0
            for s2 in range(2):
                for t in range(TPC):
                    for band in range(4):
                        cc = s2 * 4 + band
                        t64 = cc * TPC + t
                        mm = nc.tensor.matmul(
                            ps[band * G : (band + 1) * G,
                               s2 * D : (s2 + 1) * D],
                            lhsT=ohv[:, j, :, t64],
                            rhs=xring[:, (j * SGT + t64) * D
                                      : (j * SGT + t64) * D + D],
                            start=(t == 0),
                            stop=(t == TPC - 1),
                            tile_position=(0, band * G),
                            skip_group_check=True,
                        )
                        n_mm += 1
                        if n_mm == 1 and release_oh_prev:
                            # previous ring slot's oh is fully consumed
                            # once this MM (strictly later in PE FIFO
                            # order) completes
                            mm.then_inc(s_ohf, 1)
                        elif n_mm == 2 and release_strip_prev:
                            # the strip ending at the previous slot is
                            # fully consumed once this MM completes
                            mm.then_inc(s_xf, 1)
                        elif n_mm == 64:
                            mm.then_inc(s_ped, 1)

        # releases are unconditional: the first SG's "release" of its
        # non-existent predecessors contributes one spurious +1 to s_ohf
        # and s_xf each, absorbed in the consumer thresholds above
        n_peel = min(U, n_sg)
        for j in range(n_peel):
            pe_sg(j, release_strip_prev=True, peel_s=j)
        n_loop2 = (n_sg - n_peel) // U
        n_tail2 = (n_sg - n_peel) % U
        if n_loop2:
            with nc.tensor.Fori(0, n_loop2):
                for j in range(U):
                    pe_sg(j, release_strip_prev=True)
                nc.tensor.reg_add(r_xthr, r_xthr, 16)
        for j in range(n_tail2):
            pe_sg(j, release_strip_prev=True)

        # ---- ACT: flush PSUM -> out block quarters; DMA per block ----
        for s in range(n_sg):
            sub = s % BANDS
            blk = s // BANDS
            bo = (blk % 2) * CPS * D
            nc.scalar.wait_ge(s_ped, s + 1)
            if sub == 0 and blk >= 2:
                nc.scalar.wait_ge(s_st, 16 * (blk - 1))
            nc.scalar.copy(
                outb[:, bo + sub * 2 * D : bo + (sub + 1) * 2 * D],
                psr[s % U][:, :],
            ).then_inc(s_psf, 1)
            if sub == BANDS - 1 or s == n_sg - 1:
                # the block's copies must have fully drained before the
                # HWDGE doorbell lets the SDMA engines read outb
                nc.scalar.wait_ge(s_psf, s + 1)
                nc.scalar.dma_start(
                    out=OUT[:, blk * CPS * D : (blk + 1) * CPS * D],
                    in_=outb[:, bo : bo + CPS * D],
                ).then_inc(s_st, 16)
        nc.scalar.wait_ge(s_st, 16 * n_blocks)
    nc.compile()
    return nc


# --------------------------------------------------------------------------
# entry point
# --------------------------------------------------------------------------

def _run(input_np, index_np, n_segments, trace=False, trace_kwargs=None):
    per_core, in_maps, assemble = build_device_arrays(
        input_np, index_np, n_segments
    )
    nc = build_bass(per_core)
    res = run_bass_kernel_spmd(
        nc,
        in_maps,
        core_ids=list(range(N_CORES)),
        trace=trace,
        **(trace_kwargs or {}),
    )
    outs = [np.asarray(r["out"]) for r in res.results]
    return assemble(outs), res


def kernel(input, index):
    out, _ = _run(np.asarray(input), np.asarray(index), 50000)
    return out
